# revision 1
# baseline (speedup 1.0000x reference)
"""Trainium2 Bass kernel for nn_Attention_60885456388891 (gnn_message_passing).

Computation (per batch b):
  node_h = h @ W_h2node + b_h2node
  score_n[n] = sum_d tanh(p_node_feats[b,n,d] + node_h[b,d]) * w_alpha1[d]
  node_w = renorm(softmax(score_n) * att_masks)
  node_res_ = sum_n node_w[n] * node_feats[b,n,:]
  (same for relations)
  node_res = glu(cat(node_res_, rela_res_) @ W_ng + b_ng)
  rela_res = glu(cat(rela_res_, node_res) @ W_rg + b_rg)

Strategy: pure data-parallel over batch B=512 across 8 cores (64 batches/core).
Memory-bound: streams pnf/nf/prf/rf (96 MiB/core) once at DMA line rate.

Per-core pipeline (all f32):
  - rank-1 PE matmuls broadcast node_h/rela_h rows across 128 partitions into PSUM
  - DVE tensor_add: arg = feats_tile + bcast (PSUM operand)
  - ACT tanh
  - DVE tensor_tensor_reduce: scores[:, b] = sum_d tanh * w_alpha_bcast (fused)
  - batched softmax per 16-batch group (PE transposes scores cols<->rows)
  - phase C: feats chunk stationary, weight column moving -> X^T columns
    accumulate in persistent PSUM tiles (k-chunked layout feeds phase E directly)
  - phase E: GLU head via PE matmuls (k-chunked), sigmoid on ACT
"""

import numpy as np

import concourse.bass as bass
import concourse.bacc as bacc
import concourse.mybir as mybir
import concourse.tile as tile
from concourse.bass_utils import run_bass_kernel_spmd

# Problem dims (hardcoded per contract)
B, N, R, D = 512, 128, 256, 512
NCORES = 8
BS = B // NCORES          # 64 batches per core
GROUPS = 8                # softmax groups per core
G = BS // GROUPS          # 16 batches per group
PAIR = 4                  # batches per stream DMA block
KC = D // 128             # 4 k-chunks of 128
KC2 = 2 * D // 128        # 8 k-chunks for the 1024-wide GLU matmuls

F32 = mybir.dt.float32
F16 = mybir.dt.float16
I32 = mybir.dt.int32
PHASEC_DT = F32  # knob: F32 (exact) or mybir.dt.float32r (4x faster, ~1.5e-4 err)
AF = mybir.ActivationFunctionType
ALU = mybir.AluOpType
AX = mybir.AxisListType


def _ap(t):
    """Tile or AP -> AP covering the whole tile."""
    if isinstance(t, bass.AP):
        return t
    return t[:]


def _bcast_mid(ap2d, n):
    """[P, F] AP -> [P, n, F] AP with a step-0 middle dim (re-read same data)."""
    a = _ap(ap2d)
    assert len(a.ap) == 2
    return bass.AP(tensor=a.tensor, offset=a.offset, ap=[a.ap[0], [0, n], a.ap[1]])


def _rows_flat(dram_t, b0, npair, d):
    """DRAM tile [BS, d] -> AP [1, npair, d] over rows b0..b0+npair."""
    a = _ap(dram_t)
    return bass.AP(tensor=a.tensor, offset=a.offset + b0 * d,
                   ap=[[0, 1], [d, npair], [1, d]])


def _phase_a(nc, dma, dma_s, g, pools, cs_):
    """Stream pnf/prf for one group, compute score columns."""
    pnf_pool = pools["pnf_pool"]; prf_pool = pools["prf_pool"]
    hrp = pools["hrp"]; argp = pools["argp"]
    scop = pools["scop"]; bcp = pools["bcp"]
    ones16 = cs_["ones16"]; w1b = cs_["w1b"]; w2b = cs_["w2b"]
    nh_dr = cs_["nh_dr"]; rh_dr = cs_["rh_dr"]
    pnf_d = cs_["pnf_d"]; prf_d = cs_["prf_d"]

    g0 = g * G
    scores_n = scop.tile([128, G], F32, tag="sn")
    scores_r0 = scop.tile([128, G], F32, tag="sr0")
    scores_r1 = scop.tile([128, G], F32, tag="sr1")

    for j in range(G // PAIR):
        b0 = g0 + j * PAIR
        blk = b0 // PAIR
        pnf2 = pnf_pool.tile([128, PAIR, D], F32, tag="pnf2")
        dma(out=pnf2, in_=pnf_d[blk])
        prf2 = prf_pool.tile([128, PAIR, 2, D], F32, tag="prf2")
        dma(out=prf2, in_=prf_d[blk])
        nhp = hrp.tile([1, PAIR, D], F16, tag="nhp")
        dma_s(out=nhp, in_=_rows_flat(nh_dr, b0, PAIR, D))
        rhp = hrp.tile([1, PAIR, D], F16, tag="rhp")
        dma_s(out=rhp, in_=_rows_flat(rh_dr, b0, PAIR, D))
        # half-block staging keeps each engine queue free of cross-engine
        # round-trips: adds for both batches are queued ahead of the reduces.
        for hh in range(PAIR // 2):
            tans = []
            for ii in range(2):
                i = hh * 2 + ii
                bcN = bcp.tile([128, D], F32, tag="bc")
                nc.tensor.matmul(bcN, ones16, nhp[:, i, :], start=True, stop=True)
                bcR = bcp.tile([128, D], F32, tag="bc")
                nc.tensor.matmul(bcR, ones16, rhp[:, i, :], start=True, stop=True)
                argN = argp.tile([128, D], F32, tag="argN")
                nc.vector.tensor_add(argN, pnf2[:, i, :], bcN)
                argR = argp.tile([128, 2, D], F32, tag="argR")
                nc.vector.tensor_add(argR, prf2[:, i, :, :], _bcast_mid(bcR, 2))
                tans.append((argN, argR))
            for ii in range(2):
                argN, argR = tans[ii]
                nc.scalar.activation(argN, argN, AF.Tanh)
                nc.scalar.activation(argR, argR, AF.Tanh)
            for ii in range(2):
                jj = j * PAIR + hh * 2 + ii
                tN, tR = tans[ii]
                nc.vector.scalar_tensor_tensor(
                    out=tN, in0=tN, scalar=1.0, in1=w1b,
                    op0=ALU.mult, op1=ALU.mult, accum_out=scores_n[:, jj:jj + 1])
                nc.vector.scalar_tensor_tensor(
                    out=tR[:, 0, :], in0=tR[:, 0, :], scalar=1.0, in1=w2b,
                    op0=ALU.mult, op1=ALU.mult, accum_out=scores_r0[:, jj:jj + 1])
                nc.vector.scalar_tensor_tensor(
                    out=tR[:, 1, :], in0=tR[:, 1, :], scalar=1.0, in1=w2b,
                    op0=ALU.mult, op1=ALU.mult, accum_out=scores_r1[:, jj:jj + 1])
    return scores_n, scores_r0, scores_r1


def _phase_b(nc, dma, dma_s, g, pools, cs_, scores):
    """Batched masked softmax over one group; returns weight-column tiles."""
    scores_n, scores_r0, scores_r1 = scores
    smp = pools["smp"]; wcp = pools["wcp"]; mkp = pools["mkp"]
    ptp = pools["ptp"]
    ident = cs_["ident"]
    am_d = cs_["am_d"]; rm_d = cs_["rm_d"]
    g0 = g * G

    am_i = mkp.tile([G, N], I32, tag="ami")
    dma_s(out=am_i, in_=am_d[g0:g0 + G])
    am_f = mkp.tile([G, N], F32, tag="amf")
    nc.vector.tensor_copy(am_f, am_i)
    rm_i = mkp.tile([G, R], I32, tag="rmi")
    dma_s(out=rm_i, in_=rm_d[g0:g0 + G])
    rm_f = mkp.tile([G, R], F32, tag="rmf")
    nc.vector.tensor_copy(rm_f, rm_i)

    # node softmax
    sT = ptp.tile([G, N], F32, tag="fwd")
    nc.tensor.transpose(sT, scores_n, ident)
    mneg = smp.tile([G, 1], F32, tag="mneg")
    nc.vector.tensor_reduce(out=mneg, in_=sT, axis=AX.X, op=ALU.max, negate=True)
    E = smp.tile([G, N], F32, tag="E")
    nc.scalar.activation(E, sT, AF.Exp, bias=mneg)
    EM = smp.tile([G, N], F32, tag="EM")
    nc.vector.tensor_mul(EM, E, am_f)
    S = smp.tile([G, 1], F32, tag="S")
    nc.vector.reduce_sum(out=S, in_=EM, axis=AX.X)
    rS = smp.tile([G, 1], F32, tag="rS")
    nc.vector.reciprocal(rS, S)
    Wn_w = smp.tile([G, N], F32, tag="Wn")
    nc.vector.tensor_scalar_mul(Wn_w, EM, rS)
    wT = ptp.tile([128, G], F32, tag="bwd")
    nc.tensor.transpose(wT, Wn_w, ident[:G, :G])
    WnC = wcp.tile([128, G], PHASEC_DT, tag="wnc")
    nc.scalar.copy(WnC, wT)

    # rela softmax (two 128-chunks share one softmax over R=256)
    sTr = ptp.tile([G, R], F32, tag="fwd")
    nc.tensor.transpose(sTr[:, 0:128], scores_r0, ident)
    nc.tensor.transpose(sTr[:, 128:256], scores_r1, ident)
    mneg_r = smp.tile([G, 1], F32, tag="mnegr")
    nc.vector.tensor_reduce(out=mneg_r, in_=sTr, axis=AX.X, op=ALU.max, negate=True)
    Er = smp.tile([G, R], F32, tag="Er")
    nc.scalar.activation(Er, sTr, AF.Exp, bias=mneg_r)
    EMr = smp.tile([G, R], F32, tag="EMr")
    nc.vector.tensor_mul(EMr, Er, rm_f)
    Sr = smp.tile([G, 1], F32, tag="Sr")
    nc.vector.reduce_sum(out=Sr, in_=EMr, axis=AX.X)
    rSr = smp.tile([G, 1], F32, tag="rSr")
    nc.vector.reciprocal(rSr, Sr)
    Wr_w = smp.tile([G, R], F32, tag="Wr")
    nc.vector.tensor_scalar_mul(Wr_w, EMr, rSr)
    wTr0 = ptp.tile([128, G], F32, tag="bwd")
    nc.tensor.transpose(wTr0, Wr_w[:, 0:128], ident[:G, :G])
    Wr0C = wcp.tile([128, G], PHASEC_DT, tag="wr0c")
    nc.scalar.copy(Wr0C, wTr0)
    wTr1 = ptp.tile([128, G], F32, tag="bwd")
    nc.tensor.transpose(wTr1, Wr_w[:, 128:256], ident[:G, :G])
    Wr1C = wcp.tile([128, G], PHASEC_DT, tag="wr1c")
    nc.scalar.copy(Wr1C, wTr1)

    return WnC, Wr0C, Wr1C


def _phase_c(nc, dma, dma_s, g, pools, cs_, wcols):
    """Weighted sums (weight col stationary, feats moving). Out rows land at
    partition 0, ACT copies them into partition-0 staging, small DMAs scatter
    rows into X_*_sb[b]."""
    WnC, Wr0C, Wr1C = wcols
    nf_pool = pools["nf_pool"]; rf_pool = pools["rf_pool"]
    xrp = pools["xrp"]; stgp = pools["stgp"]
    X_n_sb = cs_["X_n_sb"]; X_r_sb = cs_["X_r_sb"]
    nf_d = cs_["nf_d"]; rf_d = cs_["rf_d"]
    g0 = g * G
    for j in range(G // PAIR):
        b0 = g0 + j * PAIR
        blk = b0 // PAIR
        nf2 = nf_pool.tile([128, PAIR, D], PHASEC_DT, tag="nf2")
        dma(out=nf2, in_=nf_d[blk])
        rf2 = rf_pool.tile([128, PAIR, 2, D], PHASEC_DT, tag="rf2")
        dma(out=rf2, in_=rf_d[blk])
        for h in range(PAIR // 2):
            stage_n = stgp.tile([1, 2, D], F32, tag="stn")
            stage_r = stgp.tile([1, 2, D], F32, tag="str")
            for ii in range(2):
                i = h * 2 + ii
                jj = j * PAIR + i
                xr_n = xrp.tile([1, D], F32, tag="xrow")
                nc.tensor.matmul(xr_n, WnC[:, jj:jj + 1], nf2[:, i, :],
                                 start=True, stop=True)
                nc.scalar.copy(stage_n[:, ii, :], xr_n)
                xr_r = xrp.tile([1, D], F32, tag="xrow")
                nc.tensor.matmul(xr_r, Wr0C[:, jj:jj + 1], rf2[:, i, 0, :],
                                 start=True, stop=False)
                nc.tensor.matmul(xr_r, Wr1C[:, jj:jj + 1], rf2[:, i, 1, :],
                                 start=False, stop=True)
                nc.scalar.copy(stage_r[:, ii, :], xr_r)
            dma_s(out=X_n_sb[b0 + h * 2:b0 + h * 2 + 2, :], in_=stage_n)
            dma_s(out=X_r_sb[b0 + h * 2:b0 + h * 2 + 2, :], in_=stage_r)


def build_program():
    nc = bacc.Bacc("TRN2", target_bir_lowering=False, debug=False)

    def din(name, shape, dt=F32):
        return nc.dram_tensor(name, shape, dt, kind="ExternalInput").ap()

    NBLK = BS // PAIR
    h_d = din("h", [BS, D])
    pnf_d = din("pnf", [NBLK, 128, PAIR, D])
    nf_d = din("nf", [NBLK, 128, PAIR, D], PHASEC_DT)
    prf_d = din("prf", [NBLK, 128, PAIR, 2, D])
    rf_d = din("rf", [NBLK, 128, PAIR, 2, D], PHASEC_DT)
    am_d = din("am", [BS, N], I32)
    rm_d = din("rm", [BS, R], I32)
    Wn_d = din("w_h2node", [D, D])
    bn_d = din("b_h2node", [1, D])
    Wr_d = din("w_h2rela", [D, D])
    br_d = din("b_h2rela", [1, D])
    w1b_d = din("w1b", [128, D])
    w2b_d = din("w2b", [128, D])
    Wng_d = din("w_ng", [2 * D, 2 * D])
    bng_d = din("b_ng", [1, 2 * D])
    Wrg_d = din("w_rg", [2 * D, 2 * D])
    brg_d = din("b_rg", [1, 2 * D])
    id_d = din("ident", [128, 128])
    ones_d = din("ones_row", [1, 128])
    ones16_d = din("ones16", [1, 128], F16)

    nres_d = nc.dram_tensor("node_res", [BS, D], F32, kind="ExternalOutput").ap()
    rres_d = nc.dram_tensor("rela_res", [BS, D], F32, kind="ExternalOutput").ap()

    dma = nc.sync.dma_start
    dma_s = nc.gpsimd.dma_start

    with tile.TileContext(nc) as tc:
        with (
            tc.tile_pool(name="const", bufs=1) as cp,
            tc.tile_pool(name="dscr", bufs=1, space="DRAM") as dp,
        ):
            # ---- persistent SBUF accumulators for phase C row results ----
            X_n_sb = cp.tile([BS, D], F32, tag="xnsb")
            X_r_sb = cp.tile([BS, D], F32, tag="xrsb")

            # ---- constants / weights ----
            ident = cp.tile([128, 128], F32)
            dma(out=ident, in_=id_d)
            ones_row = cp.tile([1, 128], F32)
            dma(out=ones_row, in_=ones_d)
            ones16 = cp.tile([1, 128], F16)
            dma(out=ones16, in_=ones16_d)
            w1b = cp.tile([128, D], F32)
            dma(out=w1b, in_=w1b_d)
            w2b = cp.tile([128, D], F32)
            dma(out=w2b, in_=w2b_d)

            # ---- prologue: node_h = h @ W_h2node + b, rela_h = h @ W_h2rela + b
            # Results land in DRAM scratch so rows can be re-read at partition 0.
            nh_dr = dp.tile([BS, D], F16, tag="nhdr")
            rh_dr = dp.tile([BS, D], F16, tag="rhdr")
            with (
                tc.tile_pool(name="prolsb", bufs=1) as psb,
                tc.tile_pool(name="prol", bufs=2, space="PSUM") as pp,
            ):
                Wn_sb = psb.tile([128, KC, D], F32, tag="wn")
                dma(out=Wn_sb, in_=Wn_d.rearrange("(c p) n -> p c n", p=128))
                Wr_sb = psb.tile([128, KC, D], F32, tag="wr")
                dma(out=Wr_sb, in_=Wr_d.rearrange("(c p) n -> p c n", p=128))
                bn_sb = psb.tile([1, D], F32, tag="bn")
                dma(out=bn_sb, in_=bn_d)
                br_sb = psb.tile([1, D], F32, tag="br")
                dma(out=br_sb, in_=br_d)
                h_sb = psb.tile([BS, D], F32, tag="h")
                dma(out=h_sb, in_=h_d)
                hT_sb = psb.tile([128, KC, BS], F32, tag="ht")
                for c in range(KC):
                    hT_ps = pp.tile([128, BS], F32, tag="pt")
                    nc.tensor.transpose(hT_ps, h_sb[:, c * 128:(c + 1) * 128],
                                        ident[:BS, :BS])
                    nc.scalar.copy(hT_sb[:, c, :], hT_ps)
                for dst_dr, W_sb, b_sb, tg in ((nh_dr, Wn_sb, bn_sb, "nh"),
                                               (rh_dr, Wr_sb, br_sb, "rh")):
                    ps = pp.tile([BS, D], F32, tag="pnh")
                    for c in range(KC):
                        nc.tensor.matmul(ps, hT_sb[:, c, :], W_sb[:, c, :],
                                         start=(c == 0), stop=False)
                    nc.tensor.matmul(ps, ones_row[:, :BS], b_sb,
                                     start=False, stop=True)
                    sb = psb.tile([BS, D], F32, tag=tg)
                    nc.scalar.copy(sb, ps)
                    sb16 = psb.tile([BS, D], F16, tag=tg + "16")
                    nc.vector.tensor_copy(sb16, sb)
                    dma(out=dst_dr, in_=sb16)

            # ---- main loop ----
            with (
                tc.tile_pool(name="pnf2p", bufs=3) as pnf_pool,
                tc.tile_pool(name="prf2p", bufs=3) as prf_pool,
                tc.tile_pool(name="nf2p", bufs=3) as nf_pool,
                tc.tile_pool(name="rf2p", bufs=2) as rf_pool,
                tc.tile_pool(name="hrow", bufs=2) as hrp,
                tc.tile_pool(name="args", bufs=3) as argp,
                tc.tile_pool(name="scores", bufs=2) as scop,
                tc.tile_pool(name="smax", bufs=2) as smp,
                tc.tile_pool(name="wcols", bufs=2) as wcp,
                tc.tile_pool(name="masks", bufs=1) as mkp,
                tc.tile_pool(name="stage", bufs=2) as stgp,
                tc.tile_pool(name="bcast", bufs=4, space="PSUM") as bcp,
                tc.tile_pool(name="ptrans", bufs=1, space="PSUM") as ptp,
                tc.tile_pool(name="xrow", bufs=2, space="PSUM") as xrp,
            ):
                pools = dict(
                    pnf_pool=pnf_pool, prf_pool=prf_pool, nf_pool=nf_pool,
                    rf_pool=rf_pool, hrp=hrp, argp=argp,
                    scop=scop, smp=smp, wcp=wcp, mkp=mkp, bcp=bcp, ptp=ptp,
                    xrp=xrp, stgp=stgp,
                )
                consts = dict(
                    ident=ident, ones_row=ones_row, ones16=ones16,
                    w1b=w1b, w2b=w2b,
                    nh_dr=nh_dr, rh_dr=rh_dr, X_n_sb=X_n_sb, X_r_sb=X_r_sb,
                    pnf_d=pnf_d, prf_d=prf_d, nf_d=nf_d, rf_d=rf_d,
                    am_d=am_d, rm_d=rm_d,
                )
                wcols_prev = None
                for g in range(GROUPS):
                    scores = _phase_a(nc, dma, dma_s, g, pools, consts)
                    if wcols_prev is not None:
                        _phase_c(nc, dma, dma_s, g - 1, pools, consts,
                                 wcols_prev)
                    wcols_prev = _phase_b(nc, dma, dma_s, g, pools, consts,
                                          scores)
                _phase_c(nc, dma, dma_s, GROUPS - 1, pools, consts, wcols_prev)
            # ---- phase E: GLU head ----
            with (
                tc.tile_pool(name="esb", bufs=1) as ep,
                tc.tile_pool(name="etp", bufs=2, space="PSUM") as ept,
                tc.tile_pool(name="ebp", bufs=2, space="PSUM") as epb,
            ):
                bng_sb = ep.tile([1, 2 * D], F32, tag="bng")
                dma(out=bng_sb, in_=bng_d)
                brg_sb = ep.tile([1, 2 * D], F32, tag="brg")
                dma(out=brg_sb, in_=brg_d)
                XT_sb = ep.tile([128, KC2, BS], F32, tag="xt")
                for c in range(KC):
                    tp_ps = ept.tile([128, BS], F32, tag="et")
                    nc.tensor.transpose(tp_ps, X_n_sb[:, c * 128:(c + 1) * 128],
                                        ident[:BS, :BS])
                    nc.scalar.copy(XT_sb[:, c, :], tp_ps)
                for c in range(KC):
                    tp_ps = ept.tile([128, BS], F32, tag="et")
                    nc.tensor.transpose(tp_ps, X_r_sb[:, c * 128:(c + 1) * 128],
                                        ident[:BS, :BS])
                    nc.scalar.copy(XT_sb[:, KC + c, :], tp_ps)

                # node gate: glu(cat(Xn, Xr) @ W_ng + b_ng)
                # weights stream in half-tiles so DMA overlaps the matmuls
                Wg_half = {}
                for wd, tg in ((Wng_d, "ng"), (Wrg_d, "rg")):
                    for hh in range(2):
                        wt = ep.tile([128, KC2, D], F32, tag="wbig" + str(hh))
                        dma(out=wt,
                            in_=wd[:, hh * D:(hh + 1) * D].rearrange(
                                "(c p) n -> p c n", p=128))
                        Wg_half[(tg, hh)] = wt
                ng_ps = epb.tile([BS, 2, D], F32, tag="ebig")
                for hh in range(2):
                    for c in range(KC2):
                        nc.tensor.matmul(ng_ps[:, hh, :], XT_sb[:, c, :],
                                         Wg_half[("ng", hh)][:, c, :],
                                         start=(c == 0), stop=False)
                    nc.tensor.matmul(ng_ps[:, hh, :], ones_row[:, :BS],
                                     bng_sb[:, hh * D:(hh + 1) * D],
                                     start=False, stop=True)
                sigN = ep.tile([BS, D], F32, tag="sigN")
                nc.scalar.activation(sigN, ng_ps[:, 1, :], AF.Sigmoid)
                nres_sb = ep.tile([BS, D], F32, tag="nres")
                nc.vector.tensor_mul(nres_sb, ng_ps[:, 0, :], sigN)
                dma(out=nres_d, in_=nres_sb)

                # rela gate: glu(cat(Xr, node_res) @ W_rg + b_rg)
                NT_sb = ep.tile([128, KC, BS], F32, tag="nt")
                for c in range(KC):
                    tp_ps = ept.tile([128, BS], F32, tag="et")
                    nc.tensor.transpose(tp_ps, nres_sb[:, c * 128:(c + 1) * 128],
                                        ident[:BS, :BS])
                    nc.scalar.copy(NT_sb[:, c, :], tp_ps)
                rg_ps = epb.tile([BS, 2, D], F32, tag="ebig")
                for hh in range(2):
                    for c in range(KC2):
                        lhsT = XT_sb[:, KC + c, :] if c < KC else NT_sb[:, c - KC, :]
                        nc.tensor.matmul(rg_ps[:, hh, :], lhsT,
                                         Wg_half[("rg", hh)][:, c, :],
                                         start=(c == 0), stop=False)
                    nc.tensor.matmul(rg_ps[:, hh, :], ones_row[:, :BS],
                                     brg_sb[:, hh * D:(hh + 1) * D],
                                     start=False, stop=True)
                sigR = ep.tile([BS, D], F32, tag="sigR")
                nc.scalar.activation(sigR, rg_ps[:, 1, :], AF.Sigmoid)
                rres_sb = ep.tile([BS, D], F32, tag="rres")
                nc.vector.tensor_mul(rres_sb, rg_ps[:, 0, :], sigR)
                dma(out=rres_d, in_=rres_sb)

    nc.compile()
    return nc


def make_in_maps(inputs):
    """Shard full inputs into 8 per-core input dicts (host-side layout prep only)."""
    f32 = np.float32
    h = np.ascontiguousarray(inputs["h"], dtype=f32)
    nblk = BS // PAIR

    def shuf_n(x):  # [BS,N,D] -> [NBLK,128,PAIR,D] (tile layout, contiguous DMA)
        x = np.asarray(x, dtype=f32).reshape(nblk, PAIR, N, D)
        return np.ascontiguousarray(x.transpose(0, 2, 1, 3))

    def shuf_r(x):  # [BS,R,D] -> [NBLK,128,PAIR,2,D]
        x = np.asarray(x, dtype=f32).reshape(nblk, PAIR, 2, 128, D)
        return np.ascontiguousarray(x.transpose(0, 3, 1, 2, 4))

    pnf = np.asarray(inputs["p_node_feats"], dtype=f32)
    nf = np.asarray(inputs["node_feats"], dtype=f32)
    prf = np.asarray(inputs["p_rela_feats"], dtype=f32)
    rf = np.asarray(inputs["rela_feats"], dtype=f32)
    am = np.ascontiguousarray(inputs["att_masks"], dtype=np.int32)
    rm = np.ascontiguousarray(inputs["rela_masks"], dtype=np.int32)

    w1b = np.ascontiguousarray(
        np.broadcast_to(np.asarray(inputs["w_alpha1"], dtype=f32), (128, D)))
    w2b = np.ascontiguousarray(
        np.broadcast_to(np.asarray(inputs["w_alpha2"], dtype=f32), (128, D)))
    ident = np.eye(128, dtype=f32)
    ones_row = np.ones((1, 128), dtype=f32)
    ones16 = np.ones((1, 128), dtype=np.float16)

    shared = {
        "w_h2node": np.ascontiguousarray(inputs["W_h2node"], dtype=f32),
        "b_h2node": np.asarray(inputs["b_h2node"], dtype=f32).reshape(1, D),
        "w_h2rela": np.ascontiguousarray(inputs["W_h2rela"], dtype=f32),
        "b_h2rela": np.asarray(inputs["b_h2rela"], dtype=f32).reshape(1, D),
        "w1b": w1b,
        "w2b": w2b,
        "w_ng": np.ascontiguousarray(inputs["W_ng"], dtype=f32),
        "b_ng": np.asarray(inputs["b_ng"], dtype=f32).reshape(1, 2 * D),
        "w_rg": np.ascontiguousarray(inputs["W_rg"], dtype=f32),
        "b_rg": np.asarray(inputs["b_rg"], dtype=f32).reshape(1, 2 * D),
        "ident": ident,
        "ones_row": ones_row,
        "ones16": ones16,
    }
    in_maps = []
    for c in range(NCORES):
        s = slice(c * BS, (c + 1) * BS)
        in_maps.append({
            "h": h[s], "pnf": shuf_n(pnf[s]), "nf": shuf_n(nf[s]),
            "prf": shuf_r(prf[s]), "rf": shuf_r(rf[s]),
            "am": am[s], "rm": rm[s], **shared,
        })
    return in_maps


_NC_CACHE = None
LAST_RESULTS = None  # BassKernelResults of the most recent kernel() call


def kernel(**inputs):
    global _NC_CACHE, LAST_RESULTS
    if _NC_CACHE is None:
        _NC_CACHE = build_program()
    nc = _NC_CACHE
    in_maps = make_in_maps(inputs)
    import os
    trace = os.environ.get("BASS_KERNEL_TRACE", "0") == "1"
    res = run_bass_kernel_spmd(nc, in_maps, core_ids=list(range(NCORES)),
                               trace=trace)
    LAST_RESULTS = res
    node_res = np.concatenate([r["node_res"] for r in res.results], axis=0)
    rela_res = np.concatenate([r["rela_res"] for r in res.results], axis=0)
    return node_res, rela_res



# revision 5
# speedup vs baseline: 1.9199x; 1.9199x over previous
"""Trainium2 Bass kernel for nn_Attention_60885456388891 (gnn_message_passing).

Computation (per batch b):
  node_h = h @ W_h2node + b_h2node
  score_n[n] = sum_d tanh(p_node_feats[b,n,d] + node_h[b,d]) * w_alpha1[d]
  node_w = renorm(softmax(score_n) * att_masks)
  node_res_ = sum_n node_w[n] * node_feats[b,n,:]
  (same for relations)
  node_res = glu(cat(node_res_, rela_res_) @ W_ng + b_ng)
  rela_res = glu(cat(rela_res_, node_res) @ W_rg + b_rg)

Strategy: pure data-parallel over batch B=512 across 8 cores (64 batches/core).
Memory-bound: all four big streams (pnf/nf/prf/rf) are cast to bf16 on the
host (free) and streamed once -> 48 MiB/core at DMA line rate.

Per-core pipeline (v2, bf16):
  - pnf/prf arrive D-MAJOR ([128(d-chunk), PAIR, KC, N]) so the node_h add is
    a free-dim broadcast on DVE at 2x rate (node_h kept duplicated-pair bf16)
  - tanh fused: one in-place ACT op per PAIR block per tensor
  - score matmuls use one-hot w_alpha columns [128, G] so scores accumulate
    directly in batch-major [G, N+R] PSUM (no reduce ops, no fwd transposes)
  - softmax on [G, N] rows; weights transposed back and DIAGONALIZED into
    [128, G, G] so phase C's weighted sums accumulate all G batch rows into
    one [G, D] PSUM bank -> a single copy per group
  - GLU head in bf16; node_h projection computed transposed on-chip
"""

import numpy as np
import ml_dtypes

import concourse.bass as bass
import concourse.bacc as bacc
import concourse.mybir as mybir
import concourse.tile as tile
from concourse.bass_utils import run_bass_kernel_spmd

# Problem dims (hardcoded per contract)
B, N, R, D = 512, 128, 256, 512
NCORES = 8
BS = B // NCORES          # 64 batches per core
GROUPS = 8                # softmax groups per core
G = BS // GROUPS          # 8 batches per group
PAIR = 4                  # batches per stream DMA block
NBLK = BS // PAIR         # 16 stream blocks
KC = D // 128             # 4 d-chunks of 128
KC2 = 2 * D // 128        # 8 k-chunks for the 1024-wide GLU matmuls

F32 = mybir.dt.float32
BF16 = mybir.dt.bfloat16
I32 = mybir.dt.int32
AF = mybir.ActivationFunctionType
ALU = mybir.AluOpType
AX = mybir.AxisListType

NPBF = ml_dtypes.bfloat16


def _ap(t):
    """Tile or AP -> AP covering the whole tile."""
    if isinstance(t, bass.AP):
        return t
    return t[:]


def _view(t, off_elems, dims):
    """Reshape a tile's free dims: keep partition dim, replace free AP.

    dims: list of [step, num] pairs (innermost last), offset in elements
    added to the tile's base offset.
    """
    a = _ap(t)
    return bass.AP(tensor=a.tensor, offset=a.offset + off_elems,
                   ap=[a.ap[0]] + dims)


def build_program():
    nc = bacc.Bacc("TRN2", target_bir_lowering=False, debug=False)

    def din(name, shape, dt=BF16):
        return nc.dram_tensor(name, shape, dt, kind="ExternalInput").ap()

    pnf_d = din("pnf", [NBLK, 128, PAIR, KC, N])
    prf_d = din("prf", [NBLK, 128, PAIR, KC, R])
    nf_d = din("nf", [NBLK, 128, PAIR, D])
    rf_d = din("rf", [NBLK, 128, PAIR, 2, D])
    ht_d = din("ht", [128, KC, BS])
    wnt_d = din("wnt", [128, KC, D])
    wrt_d = din("wrt", [128, KC, D])
    bnt_d = din("bnt", [1, KC, 128])
    brt_d = din("brt", [1, KC, 128])
    w1m_d = din("w1m", [128, G, KC, G])
    w2m_d = din("w2m", [128, G, KC, G])
    amb_d = din("amb", [BS, N])
    rmb_d = din("rmb", [BS, R])
    wng_d = din("wng", [128, KC2, 2, D])
    wrg_d = din("wrg", [128, KC2, 2, D])
    bng_d = din("bng", [1, 2, D])
    brg_d = din("brg", [1, 2, D])
    id_d = din("ident", [128, 128])
    ones_d = din("ones_row", [1, 128])

    nres_d = nc.dram_tensor("node_res", [BS, D], F32, kind="ExternalOutput").ap()
    rres_d = nc.dram_tensor("rela_res", [BS, D], F32, kind="ExternalOutput").ap()

    dma = nc.sync.dma_start
    dma_s = nc.gpsimd.dma_start

    with tile.TileContext(nc) as tc:
        with tc.tile_pool(name="const", bufs=1) as cp:
            # ---- constants / weights (resident all kernel) ----
            ident = cp.tile([128, 128], BF16)
            dma(out=ident, in_=id_d)
            ones_row = cp.tile([1, 128], BF16)
            dma(out=ones_row, in_=ones_d)
            w1m_sb = cp.tile([128, G, KC, G], BF16)
            dma(out=w1m_sb, in_=w1m_d)
            w2m_sb = cp.tile([128, G, KC, G], BF16)
            dma(out=w2m_sb, in_=w2m_d)
            wng_sb = cp.tile([128, KC2, 2, D], BF16)
            dma(out=wng_sb, in_=wng_d)
            wrg_sb = cp.tile([128, KC2, 2, D], BF16)
            dma(out=wrg_sb, in_=wrg_d)
            bng_sb = cp.tile([1, 2, D], BF16)
            dma(out=bng_sb, in_=bng_d)
            brg_sb = cp.tile([1, 2, D], BF16)
            dma(out=brg_sb, in_=brg_d)

            # persistent result tiles
            # X rows per group live on partitions 0..G-1, group on free dim
            X_n_sb = cp.tile([G, GROUPS, D], BF16, tag="xnsb")
            X_r_sb = cp.tile([G, GROUPS, D], BF16, tag="xrsb")
            # transposed X columns for the GLU head: chunks 0..3 = Xn, 4..7 = Xr
            catXT = cp.tile([128, KC2, GROUPS, G], BF16, tag="catxt")
            # duplicated-pair transposed projections (bias source for the adds)
            nhT2 = cp.tile([128, KC, BS, 2], BF16, tag="nht2")
            rhT2 = cp.tile([128, KC, BS, 2], BF16, tag="rht2")

            # ---- prologue: nhT2 = (h @ W_h2node + b).T duplicated, same rela
            with (
                tc.tile_pool(name="prolsb", bufs=1) as psb,
                tc.tile_pool(name="prolps", bufs=2, space="PSUM") as pp,
            ):
                ht_sb = psb.tile([128, KC, BS], BF16, tag="ht")
                dma(out=ht_sb, in_=ht_d)
                wnt_sb = psb.tile([128, KC, D], BF16, tag="wnt")
                dma(out=wnt_sb, in_=wnt_d)
                wrt_sb = psb.tile([128, KC, D], BF16, tag="wrt")
                dma(out=wrt_sb, in_=wrt_d)
                bnt_sb = psb.tile([1, KC, 128], BF16, tag="bnt")
                dma(out=bnt_sb, in_=bnt_d)
                brt_sb = psb.tile([1, KC, 128], BF16, tag="brt")
                dma(out=brt_sb, in_=brt_d)
                for w_sb, b_sb, dst in ((wnt_sb, bnt_sb, nhT2),
                                        (wrt_sb, brt_sb, rhT2)):
                    for dc in range(KC):
                        ps = pp.tile([128, BS], F32, tag="pnh")
                        for kc in range(KC):
                            nc.tensor.matmul(
                                ps, w_sb[:, kc, dc * 128:(dc + 1) * 128],
                                ht_sb[:, kc, :],
                                start=(kc == 0), stop=False)
                        nc.tensor.matmul(ps, b_sb[:, dc, :],
                                         ones_row[:, :BS],
                                         start=False, stop=True)
                        # write twice (dup pair) with stride-2 free APs
                        for k in range(2):
                            outap = _view(dst, dc * BS * 2 + k, [[2, BS]])
                            nc.scalar.copy(outap, ps)

            # ---- main loop ----
            with (
                tc.tile_pool(name="pnf2p", bufs=3) as pnfp,
                tc.tile_pool(name="prf2p", bufs=3) as prfp,
                tc.tile_pool(name="nf2p", bufs=3) as nfp,
                tc.tile_pool(name="rf2p", bufs=2) as rfp,
                tc.tile_pool(name="args", bufs=2) as argp,
                tc.tile_pool(name="smax", bufs=2) as smp,
                tc.tile_pool(name="wcd", bufs=2) as wcdp,
                tc.tile_pool(name="masks", bufs=2) as mkp,
                tc.tile_pool(name="scps", bufs=2, space="PSUM") as scp,
                tc.tile_pool(name="xnps", bufs=2, space="PSUM") as xnp,
                tc.tile_pool(name="xrps", bufs=2, space="PSUM") as xrp,
                tc.tile_pool(name="ptrans", bufs=2, space="PSUM") as ptp,
            ):
                pools = dict(pnfp=pnfp, prfp=prfp, nfp=nfp, rfp=rfp,
                             argp=argp, smp=smp, wcdp=wcdp, mkp=mkp,
                             scp=scp, xnp=xnp, xrp=xrp, ptp=ptp)
                cs = dict(ident=ident, ones_row=ones_row,
                          w1m_sb=w1m_sb, w2m_sb=w2m_sb,
                          nhT2=nhT2, rhT2=rhT2,
                          X_n_sb=X_n_sb, X_r_sb=X_r_sb, catXT=catXT,
                          pnf_d=pnf_d, prf_d=prf_d, nf_d=nf_d, rf_d=rf_d,
                          amb_d=amb_d, rmb_d=rmb_d)
                wprev = None
                for g in range(GROUPS):
                    sc = _phase_a(nc, dma, dma_s, g, pools, cs)
                    if wprev is not None:
                        _phase_c(nc, dma, dma_s, g - 1, pools, cs, wprev)
                    wprev = _phase_b(nc, dma, dma_s, g, pools, cs, sc)
                _phase_c(nc, dma, dma_s, GROUPS - 1, pools, cs, wprev)

            # ---- GLU head ----
            with (
                tc.tile_pool(name="esb", bufs=1) as ep,
                tc.tile_pool(name="etp", bufs=2, space="PSUM") as ept,
                tc.tile_pool(name="ebp", bufs=2, space="PSUM") as epb,
            ):
                # node gate: glu(cat(Xn, Xr) @ W_ng + b_ng)
                ng_ps = epb.tile([BS, 2, D], F32, tag="ebig")
                for hh in range(2):
                    for c in range(KC2):
                        nc.tensor.matmul(ng_ps[:, hh, :], catXT[:, c, :, :],
                                         wng_sb[:, c, hh, :],
                                         start=(c == 0), stop=False)
                    nc.tensor.matmul(ng_ps[:, hh, :], ones_row[:, :BS],
                                     bng_sb[:, hh, :], start=False, stop=True)
                sigN = ep.tile([BS, D], F32, tag="sigN")
                nc.scalar.activation(sigN, ng_ps[:, 1, :], AF.Sigmoid)
                nres_sb = ep.tile([BS, D], F32, tag="nres")
                nc.vector.tensor_mul(nres_sb, ng_ps[:, 0, :], sigN)
                dma(out=nres_d, in_=nres_sb)

                # rela gate: glu(cat(Xr, node_res) @ W_rg + b_rg)
                nres_bf = ep.tile([BS, D], BF16, tag="nresbf")
                nc.vector.tensor_copy(nres_bf, nres_sb)
                NT = ep.tile([128, KC, BS], BF16, tag="nt")
                for c in range(KC):
                    tp = ept.tile([128, BS], BF16, tag="et")
                    nc.tensor.transpose(tp, nres_bf[:, c * 128:(c + 1) * 128],
                                        ident[:BS, :BS])
                    nc.scalar.copy(NT[:, c, :], tp)
                rg_ps = epb.tile([BS, 2, D], F32, tag="ebig")
                for hh in range(2):
                    for c in range(KC2):
                        lhsT = (catXT[:, KC + c, :, :] if c < KC
                                else NT[:, c - KC, :])
                        nc.tensor.matmul(rg_ps[:, hh, :], lhsT,
                                         wrg_sb[:, c, hh, :],
                                         start=(c == 0), stop=False)
                    nc.tensor.matmul(rg_ps[:, hh, :], ones_row[:, :BS],
                                     brg_sb[:, hh, :], start=False, stop=True)
                sigR = ep.tile([BS, D], F32, tag="sigR")
                nc.scalar.activation(sigR, rg_ps[:, 1, :], AF.Sigmoid)
                rres_sb = ep.tile([BS, D], F32, tag="rres")
                nc.vector.tensor_mul(rres_sb, rg_ps[:, 0, :], sigR)
                dma(out=rres_d, in_=rres_sb)

    nc.compile()
    return nc


def _phase_a(nc, dma, dma_s, g, pools, cs):
    """Stream pnf/prf (d-major bf16), add projections, tanh, score matmuls.

    Scores accumulate batch-major into one [G, N+R] PSUM tile via one-hot
    w_alpha column weights."""
    pnfp = pools["pnfp"]; prfp = pools["prfp"]; argp = pools["argp"]
    scp = pools["scp"]
    w1m_sb = cs["w1m_sb"]; w2m_sb = cs["w2m_sb"]
    nhT2 = cs["nhT2"]; rhT2 = cs["rhT2"]
    pnf_d = cs["pnf_d"]; prf_d = cs["prf_d"]

    g0 = g * G
    sc = scp.tile([G, N + R], F32, tag="sc")
    nmm = 0
    rmm = 0
    for j in range(G // PAIR):
        b0 = g0 + j * PAIR
        blk = b0 // PAIR
        pnf2 = pnfp.tile([128, PAIR, KC, N], BF16, tag="pnf2")
        dma(out=pnf2, in_=pnf_d[blk])
        prf2 = prfp.tile([128, PAIR, KC, R], BF16, tag="prf2")
        dma(out=prf2, in_=prf_d[blk])
        argN = argp.tile([128, PAIR, KC, N], BF16, tag="argN")
        argR = argp.tile([128, PAIR, KC, R], BF16, tag="argR")
        # adds: [128, PAIR, X/2, 2]-viewed, node_h broadcast along free dim
        # (dup-pair innermost keeps the packed-2byte fast path on DVE)
        for c in range(KC):
            outN = _view(argN, c * N, [[KC * N, PAIR], [2, N // 2], [1, 2]])
            inN = _view(pnf2, c * N, [[KC * N, PAIR], [2, N // 2], [1, 2]])
            bcN = _view(nhT2, c * BS * 2 + b0 * 2,
                        [[2, PAIR], [0, N // 2], [1, 2]])
            nc.vector.tensor_add(outN, inN, bcN)
            outR = _view(argR, c * R, [[KC * R, PAIR], [2, R // 2], [1, 2]])
            inR = _view(prf2, c * R, [[KC * R, PAIR], [2, R // 2], [1, 2]])
            bcR = _view(rhT2, c * BS * 2 + b0 * 2,
                        [[2, PAIR], [0, R // 2], [1, 2]])
            nc.vector.tensor_add(outR, inR, bcR)
        nc.scalar.activation(argN, argN, AF.Tanh)
        nc.scalar.activation(argR, argR, AF.Tanh)
        for i in range(PAIR):
            jj = j * PAIR + i
            for c in range(KC):
                nc.tensor.matmul(sc[:, 0:N], w1m_sb[:, jj, c, :],
                                 argN[:, i, c, :],
                                 start=(nmm == 0), stop=(nmm == G * KC - 1),
                                 skip_group_check=True)
                nmm += 1
                # start only on the very first node MM: its start marks the
                # whole 2KB zero region (incl. the rela columns), so rela's
                # first write lands as overwrite via the pending-zero bits.
                nc.tensor.matmul(sc[:, N:N + R], w2m_sb[:, jj, c, :],
                                 argR[:, i, c, :],
                                 start=False, stop=(rmm == G * KC - 1),
                                 skip_group_check=True)
                rmm += 1
    return sc


def _phase_b(nc, dma, dma_s, g, pools, cs, sc):
    """Masked softmax on batch-major scores; produce diagonalized weight
    tiles [128, G, G] for phase C."""
    smp = pools["smp"]; wcdp = pools["wcdp"]; mkp = pools["mkp"]
    ptp = pools["ptp"]
    ident = cs["ident"]
    amb_d = cs["amb_d"]; rmb_d = cs["rmb_d"]
    g0 = g * G

    am_t = mkp.tile([G, N], BF16, tag="amt")
    dma_s(out=am_t, in_=amb_d[g0:g0 + G])
    rm_t = mkp.tile([G, R], BF16, tag="rmt")
    dma_s(out=rm_t, in_=rmb_d[g0:g0 + G])

    out_cols = []
    for (c0, c1, m_t, nseg) in ((0, N, am_t, 1), (N, N + R, rm_t, 2)):
        width = c1 - c0
        mneg = smp.tile([G, 1], F32, tag=f"mneg{c0}")
        nc.vector.tensor_reduce(out=mneg, in_=sc[:, c0:c1], axis=AX.X,
                                op=ALU.max, negate=True)
        E = smp.tile([G, width], BF16, tag=f"E{c0}")
        nc.scalar.activation(E, sc[:, c0:c1], AF.Exp, bias=mneg)
        EM = smp.tile([G, width], BF16, tag=f"EM{c0}")
        nc.vector.tensor_mul(EM, E, m_t)
        S = smp.tile([G, 1], F32, tag=f"S{c0}")
        nc.vector.reduce_sum(out=S, in_=EM, axis=AX.X)
        rS = smp.tile([G, 1], F32, tag=f"rS{c0}")
        nc.vector.reciprocal(rS, S)
        W_w = smp.tile([G, width], BF16, tag=f"W{c0}")
        nc.vector.tensor_scalar_mul(W_w, EM, rS)
        for s in range(nseg):
            wT = ptp.tile([128, G], BF16, tag="wT")
            nc.tensor.transpose(wT, W_w[:, s * 128:(s + 1) * 128],
                                ident[:G, :G])
            WCd = wcdp.tile([128, G, G], BF16, tag=f"wcd{c0}_{s}")
            nc.vector.memset(WCd, 0.0)
            diag = _view(WCd, 0, [[G + 1, G]])
            nc.vector.tensor_copy(diag, wT)
            out_cols.append(WCd)
    return out_cols  # [WnCd, Wr0Cd, Wr1Cd]


def _phase_c(nc, dma, dma_s, g, pools, cs, wcols):
    """Weighted sums: diagonalized weights let all G batch rows accumulate
    into one [G, D] PSUM tile; single copy lands them in X_*_sb, then
    per-group transposes build catXT for the GLU."""
    WnCd, Wr0Cd, Wr1Cd = wcols
    nfp = pools["nfp"]; rfp = pools["rfp"]
    xnp = pools["xnp"]; xrp = pools["xrp"]; ptp = pools["ptp"]
    X_n_sb = cs["X_n_sb"]; X_r_sb = cs["X_r_sb"]; catXT = cs["catXT"]
    ident = cs["ident"]
    nf_d = cs["nf_d"]; rf_d = cs["rf_d"]
    g0 = g * G

    Xn_ps = xnp.tile([G, D], F32, tag="xn")
    Xr_ps = xrp.tile([G, D], F32, tag="xr")
    for j in range(G // PAIR):
        b0 = g0 + j * PAIR
        blk = b0 // PAIR
        nf2 = nfp.tile([128, PAIR, D], BF16, tag="nf2")
        dma(out=nf2, in_=nf_d[blk])
        rf2 = rfp.tile([128, PAIR, 2, D], BF16, tag="rf2")
        dma(out=rf2, in_=rf_d[blk])
        for i in range(PAIR):
            jj = j * PAIR + i
            nc.tensor.matmul(Xn_ps, WnCd[:, jj, :], nf2[:, i, :],
                             start=(jj == 0), stop=(jj == G - 1))
            nc.tensor.matmul(Xr_ps, Wr0Cd[:, jj, :], rf2[:, i, 0, :],
                             start=(jj == 0), stop=False)
            nc.tensor.matmul(Xr_ps, Wr1Cd[:, jj, :], rf2[:, i, 1, :],
                             start=False, stop=(jj == G - 1))
    nc.vector.tensor_copy(X_n_sb[:, g, :], Xn_ps)
    nc.vector.tensor_copy(X_r_sb[:, g, :], Xr_ps)
    for c in range(KC):
        tpn = ptp.tile([128, G], BF16, tag="wT")
        nc.tensor.transpose(tpn, X_n_sb[:, g, c * 128:(c + 1) * 128],
                            ident[:G, :G])
        nc.scalar.copy(catXT[:, c, g, :], tpn)
        tpr = ptp.tile([128, G], BF16, tag="wT")
        nc.tensor.transpose(tpr, X_r_sb[:, g, c * 128:(c + 1) * 128],
                            ident[:G, :G])
        nc.scalar.copy(catXT[:, KC + c, g, :], tpr)


def make_in_maps(inputs):
    """Shard full inputs into 8 per-core input dicts (host-side layout and
    dtype prep only; all math runs on device)."""
    f32 = np.float32

    def bf(x):
        return np.ascontiguousarray(np.asarray(x, dtype=f32).astype(NPBF))

    h = np.asarray(inputs["h"], dtype=f32)
    pnf = np.asarray(inputs["p_node_feats"], dtype=f32)
    nf = np.asarray(inputs["node_feats"], dtype=f32)
    prf = np.asarray(inputs["p_rela_feats"], dtype=f32)
    rf = np.asarray(inputs["rela_feats"], dtype=f32)
    am = np.asarray(inputs["att_masks"])
    rm = np.asarray(inputs["rela_masks"])

    def shuf_p(x, L):  # [BS, L, D] -> [NBLK, 128, PAIR, KC, L] (d-major)
        x = x.reshape(NBLK, PAIR, L, KC, 128)
        return bf(x.transpose(0, 4, 1, 3, 2))

    def shuf_n(x):  # [BS, N, D] -> [NBLK, 128, PAIR, D]
        x = x.reshape(NBLK, PAIR, N, D)
        return bf(x.transpose(0, 2, 1, 3))

    def shuf_r(x):  # [BS, R, D] -> [NBLK, 128, PAIR, 2, D]
        x = x.reshape(NBLK, PAIR, 2, 128, D)
        return bf(x.transpose(0, 3, 1, 2, 4))

    def onehot(w):  # [D] -> [128, G, KC, G]
        wr = np.asarray(w, dtype=f32).reshape(KC, 128)
        m = np.zeros((128, G, KC, G), dtype=f32)
        for jj in range(G):
            m[:, jj, :, jj] = wr.T
        return bf(m)

    wng = np.asarray(inputs["W_ng"], dtype=f32)
    wrg = np.asarray(inputs["W_rg"], dtype=f32)
    shared = {
        "wnt": bf(np.asarray(inputs["W_h2node"], dtype=f32)
                  .reshape(KC, 128, D).transpose(1, 0, 2)),
        "wrt": bf(np.asarray(inputs["W_h2rela"], dtype=f32)
                  .reshape(KC, 128, D).transpose(1, 0, 2)),
        "bnt": bf(np.asarray(inputs["b_h2node"], dtype=f32)
                  .reshape(1, KC, 128)),
        "brt": bf(np.asarray(inputs["b_h2rela"], dtype=f32)
                  .reshape(1, KC, 128)),
        "w1m": onehot(inputs["w_alpha1"]),
        "w2m": onehot(inputs["w_alpha2"]),
        "wng": bf(wng.reshape(KC2, 128, 2, D).transpose(1, 0, 2, 3)),
        "wrg": bf(wrg.reshape(KC2, 128, 2, D).transpose(1, 0, 2, 3)),
        "bng": bf(np.asarray(inputs["b_ng"], dtype=f32).reshape(1, 2, D)),
        "brg": bf(np.asarray(inputs["b_rg"], dtype=f32).reshape(1, 2, D)),
        "ident": bf(np.eye(128, dtype=f32)),
        "ones_row": bf(np.ones((1, 128), dtype=f32)),
    }
    in_maps = []
    for c in range(NCORES):
        s = slice(c * BS, (c + 1) * BS)
        in_maps.append({
            "pnf": shuf_p(pnf[s], N), "prf": shuf_p(prf[s], R),
            "nf": shuf_n(nf[s]), "rf": shuf_r(rf[s]),
            "ht": bf(h[s].reshape(BS, KC, 128).transpose(2, 1, 0)),
            "amb": bf(am[s]), "rmb": bf(rm[s]),
            **shared,
        })
    return in_maps


_NC_CACHE = None
LAST_RESULTS = None  # BassKernelResults of the most recent kernel() call


def kernel(**inputs):
    global _NC_CACHE, LAST_RESULTS
    if _NC_CACHE is None:
        _NC_CACHE = build_program()
    nc = _NC_CACHE
    in_maps = make_in_maps(inputs)
    import os
    trace = os.environ.get("BASS_KERNEL_TRACE", "0") == "1"
    res = run_bass_kernel_spmd(nc, in_maps, core_ids=list(range(NCORES)),
                               trace=trace)
    LAST_RESULTS = res
    node_res = np.concatenate([r["node_res"] for r in res.results], axis=0)
    rela_res = np.concatenate([r["rela_res"] for r in res.results], axis=0)
    return node_res, rela_res
